# revision 1
# baseline (speedup 1.0000x reference)
"""Trainium2 Bass kernel for nn_ContextualAttention.

Per sample b (one per NeuronCore):
    X   = foreground[b]               # [256, 4096]  (channels x pixels)
    K   = (X + eps).T, L2-normalized rows          # [4096, 256]
    S   = K @ X                        # [4096(k), 4096(p)] scores
    A   = softmax(S, axis=k)
    out = K.T @ A                      # [256, 4096]

On-chip layout (per core):
    X_sb    [128, 2, HW]   channels on partitions (2 chunks of 128), f32r
    Khat    [128, KT, 256] k on partitions (KT tiles of 128), channels free
    scores tile [128(k), 512(p)] in PSUM <- mm over c (2 accum steps)
    E = exp(recip_n[k] * s)  via ACT with per-partition scale, PSUM->SBUF
    out_psum [128(c), 512(p)] += Khat_tile.T @ E   (accum over k tiles)
    Z [128, 512] += ones128.T @ E   (denominator, replicated on all
        partitions so 1/Z is a full-width DVE op and needs no broadcast)
    out = (ACT-evacuated out_psum) * (1/Z) on DVE, then DMA to DRAM.
    The mm2/Z group for k-chunk kc issues after mm1 of kc+1 (software
    pipeline) so each exp has a full k-step of ACT slack.

Matmuls run in float32r (full-rate fp32 on the PE); every tile feeding a
matmul is produced with dtype float32r to satisfy walrus's rounding check.

eps=1e-7 is dropped: its effect on the output is O(1e-7) relative, far
below matmul precision.
"""

import numpy as np
from contextlib import ExitStack

import concourse.bass as bass
import concourse.tile as tile
from concourse import mybir
from concourse.bass_utils import run_bass_kernel_spmd
from concourse.masks import make_identity

F32 = mybir.dt.float32
F32R = mybir.dt.float32r
AF = mybir.ActivationFunctionType
ALU = mybir.AluOpType

CH = 256     # channels
P = 128      # partitions
PT = 512     # pixel-tile width (matmul moving dim / psum bank)
N_CORES = 8


def _emit(tc: "tile.TileContext", x: bass.AP, out: bass.AP, hw: int):
    nc = tc.nc
    CC = CH // P          # channel chunks (2)
    KT = hw // P          # k tiles (32)
    NPT = hw // PT        # pixel tiles (8)

    with ExitStack() as ctx:
        const = ctx.enter_context(tc.tile_pool(name="const", bufs=1))
        sb = ctx.enter_context(tc.tile_pool(name="sb", bufs=1))

        X = sb.tile([P, CC, hw], F32R, tag="X")
        XT = sb.tile([P, KT, CH], F32, tag="XT")
        Khat = sb.tile([P, KT, CH], F32R, tag="Khat")
        n2 = sb.tile([P, KT], F32, tag="n2")
        recip_n = sb.tile([P, KT], F32, tag="recip_n")

        ident = const.tile([P, P], F32, tag="ident")
        ones_f = const.tile([P, P], F32, tag="ones_f")
        ones128 = const.tile([P, P], F32R, tag="ones128")
        make_identity(nc, ident)
        nc.vector.memset(ones_f, 1.0)
        with nc.allow_low_precision(reason="f32r matmul operand prep"):
            nc.vector.tensor_copy(ones128, ones_f)

        # ---- load X: [256, hw] -> [128, cc, hw] ----
        # cc-inner order so early k-tiles (which need both channel halves)
        # arrive first and transposes can start sooner.
        # First slices small so the first transposes start ASAP.
        bounds = [0, hw // 16, hw // 4, hw // 2, 3 * hw // 4, hw]
        for lo, hi in zip(bounds, bounds[1:]):
            for cc in range(CC):
                nc.sync.dma_start(
                    out=X[:, cc, lo:hi],
                    in_=x[cc * P:(cc + 1) * P, lo:hi].bitcast(F32R),
                )

        # ---- setup: transpose X -> XT; n2 = row sumsq; Khat = XT/|XT| ----
        with tc.tile_pool(name="tpsum", bufs=2, space="PSUM") as tpsum, \
             tc.tile_pool(name="tsq", bufs=2) as tsq:
            for kt in range(KT):
                pt_ = tpsum.tile([P, CH], F32, tag="t")
                for cc in range(CC):
                    nc.tensor.transpose(
                        pt_[:, cc * P:(cc + 1) * P],
                        X[:, cc, kt * P:(kt + 1) * P].bitcast(F32),
                        ident,
                    )
                nc.scalar.copy(XT[:, kt, :], pt_)
                sq = tsq.tile([P, CH], F32, tag="sq")
                nc.scalar.activation(
                    sq,
                    XT[:, kt, :],
                    AF.Square,
                    accum_out=n2[:, kt:kt + 1],
                )
            # recip_n = 1/sqrt(n2)
            nc.scalar.sqrt(n2, n2)
            nc.vector.reciprocal(recip_n, n2)
            # Khat = XT * recip_n (per-partition scalar), rounded to f32r
            with nc.allow_low_precision(reason="f32r matmul operand prep"):
                for kt in range(KT):
                    nc.vector.tensor_scalar_mul(
                        out=Khat[:, kt, :],
                        in0=XT[:, kt, :],
                        scalar1=recip_n[:, kt:kt + 1],
                    )

        # ---- main: per pixel-tile flash attention ----
        with tc.tile_pool(name="ps", bufs=4, space="PSUM") as ps_pool, \
             tc.tile_pool(name="acc", bufs=3, space="PSUM") as acc_pool, \
             tc.tile_pool(name="zps", bufs=1, space="PSUM") as zps_pool, \
             tc.tile_pool(name="ework", bufs=6) as e_pool, \
             tc.tile_pool(name="owork", bufs=4) as o_pool, \
             tc.tile_pool(name="zwork", bufs=2) as z_pool:
            for pt in range(NPT):
                out_ps = [
                    acc_pool.tile([P, PT], F32, tag="acc", name=f"out_ps{cc}")
                    for cc in range(CC)
                ]
                z_ps = zps_pool.tile([P, PT], F32, tag="z")

                def mm2_group(kc, e_sb):
                    # out[c, p] += Khat[k, c].T @ E
                    for cc in range(CC):
                        nc.tensor.matmul(
                            out_ps[cc],
                            lhsT=Khat[:, kc, cc * P:(cc + 1) * P],
                            rhs=e_sb,
                            start=(kc == 0),
                            stop=(kc == KT - 1),
                        )

                def z_group(kc, e_sb):
                    # Z[p] += ones.T @ E; ones128 keeps Z replicated on all
                    # partitions so the epilogue reciprocal is full-width.
                    nc.tensor.matmul(
                        z_ps,
                        lhsT=ones128,
                        rhs=e_sb,
                        start=(kc == 0),
                        stop=(kc == KT - 1),
                    )

                # Software-pipelined: mm2 for k-chunk kc issues after mm1 of
                # kc+1, giving each exp a full k-step of slack on ACT.
                pending = None
                for kc in range(KT):
                    # scores[k, p] = sum_c X[c, k] * X[c, p]
                    s_ps = ps_pool.tile([P, PT], F32, tag="ps")
                    for cc in range(CC):
                        nc.tensor.matmul(
                            s_ps,
                            lhsT=X[:, cc, kc * P:(kc + 1) * P],
                            rhs=X[:, cc, pt * PT:(pt + 1) * PT],
                            start=(cc == 0),
                            stop=(cc == CC - 1),
                        )
                    # E = exp(recip_n[k] * s)
                    e_sb = e_pool.tile([P, PT], F32R, tag="e")
                    nc.scalar.activation(
                        e_sb, s_ps, AF.Exp, scale=recip_n[:, kc:kc + 1],
                    )
                    if pending is not None:
                        mm2_group(*pending)
                        z_group(*pending)
                    pending = (kc, e_sb)
                mm2_group(*pending)
                z_group(*pending)
                # Epilogue. Evacuate out_ps via ACT (runs parallel with the
                # DVE reciprocal) so the PSUM banks release quickly for the
                # next pixel-tile; then scale on DVE from SBUF.
                rz_sb = z_pool.tile([P, PT], F32, tag="rz")
                nc.vector.reciprocal(rz_sb, z_ps)
                for cc in range(CC):
                    o_sb = o_pool.tile([P, PT], F32, tag="o", name=f"o{cc}")
                    nc.scalar.copy(o_sb, out_ps[cc])
                    nc.vector.tensor_mul(o_sb, o_sb, rz_sb)
                    nc.sync.dma_start(
                        out=out[cc * P:(cc + 1) * P, pt * PT:(pt + 1) * PT],
                        in_=o_sb,
                    )


def _legalize_single_wait(nc: bass.Bass) -> None:
    """The walrus build in this container accepts at most ONE sync-wait per
    instruction ("Too many sync wait commands"); Tile emits instructions with
    one wait per outstanding producer. Hoist extra waits onto injected
    same-engine NOPs placed immediately before the instruction — identical
    blocking semantics, one wait each."""
    for fn in nc.m.functions:
        for bb in fn.blocks:
            new = []
            changed = False
            for inst in bb.instructions:
                if (
                    isinstance(inst, mybir.InstISA)
                    and inst.engine == mybir.EngineType.Pool
                ):
                    # Tail-of-kernel semaphore RANGE_CLEAR on GpSimd; this
                    # walrus build rejects its encoding ("ISA wrong length").
                    # Semaphores are re-initialized by the runtime at
                    # execution start, so the in-kernel clear is redundant.
                    # (DVE InstISA ops — e.g. tensor_tensor_reduce — are real
                    # compute and must be kept.)
                    changed = True
                    continue
                si = inst.sync_info
                if si is not None and si.on_wait is not None and len(si.on_wait) > 1:
                    waits = list(si.on_wait)
                    for j, w in enumerate(waits[:-1]):
                        nop = mybir.InstNoOp(
                            name=f"{inst.name}-xw{j}",
                            engine=inst.engine,
                            sync_info=mybir.SyncInfo(on_wait=[w], on_update=[]),
                            bass_nofuse=True,
                        )
                        new.append(nop)
                    si.on_wait = [waits[-1]]
                    changed = True
                new.append(inst)
            if changed:
                bb.instructions = new


def build_nc(hw: int = 4096, legalize: bool = True) -> bass.Bass:
    nc = bass.Bass()
    x = nc.dram_tensor("x", [CH, hw], F32, kind="ExternalInput")
    out = nc.dram_tensor("out", [CH, hw], F32, kind="ExternalOutput")
    with tile.TileContext(nc) as tc:
        _emit(tc, x[:], out[:], hw)
    if legalize:
        _legalize_single_wait(nc)
    return nc


_nc_cache: dict = {}


def kernel(foreground: np.ndarray) -> np.ndarray:
    fg = np.ascontiguousarray(np.asarray(foreground, dtype=np.float32))
    bs, ch, h, w = fg.shape
    assert bs == N_CORES and ch == CH
    hw = h * w
    if hw not in _nc_cache:
        _nc_cache[hw] = build_nc(hw)
    nc = _nc_cache[hw]
    in_maps = [{"x": fg[i].reshape(ch, hw)} for i in range(bs)]
    res = run_bass_kernel_spmd(nc, in_maps, core_ids=list(range(bs)))
    return np.stack(
        [np.asarray(res.results[i]["out"]).reshape(ch, h, w) for i in range(bs)]
    )



# revision 4
# speedup vs baseline: 1.1292x; 1.1292x over previous
"""Trainium2 Bass kernel for nn_ContextualAttention.

Per sample b (one per NeuronCore):
    X   = foreground[b]               # [256, 4096]  (channels x pixels)
    K   = (X + eps).T, L2-normalized rows          # [4096, 256]
    S   = K @ X                        # [4096(k), 4096(p)] scores
    A   = softmax(S, axis=k)
    out = K.T @ A                      # [256, 4096]

On-chip layout (per core):
    X_sb    [128, 2, HW]   channels on partitions (2 chunks of 128), f32r
    Khat_aug [128, KT, 257] k on partitions, channels free, 257th col = 1.0
    scores tile [128(k), 512(p)] in PSUM <- mm over c (2 accum steps)
    E = exp(recip_n[k] * s)  via ACT with per-partition scale, PSUM->SBUF
    mm2 SWAPPED: outT[p, c] += E_chunk.T @ Khat_aug  (E is the stationary
        operand, Khat_aug the moving one).  The ones column of Khat_aug
        makes column 256 of outT the softmax denominator Z for free:
        moving length 257 instead of a separate 512-wide Z matmul.
    Epilogue per p-chunk: rz = 1/outT[:,256] (DVE), outT[:, :256]*rz with a
        per-partition scalar multiply (DVE), DMA to DRAM as out^T [hw, 256].
    Host un-transposes the [hw, 256] result to [256, h, w].

Matmuls run in float32r (full-rate fp32 on the PE); every tile feeding a
matmul is produced with dtype float32r to satisfy walrus's rounding check.

eps=1e-7 is dropped: its effect on the output is O(1e-7) relative, far
below matmul precision.
"""

import numpy as np
from contextlib import ExitStack

import concourse.bass as bass
import concourse.tile as tile
from concourse import mybir
from concourse.bass_utils import run_bass_kernel_spmd
from concourse.masks import make_identity

F32 = mybir.dt.float32
F32R = mybir.dt.float32r
AF = mybir.ActivationFunctionType
ALU = mybir.AluOpType

CH = 256     # channels
P = 128      # partitions
PT = 512     # pixel-tile width (matmul moving dim / psum bank)
N_CORES = 8


def _emit(tc: "tile.TileContext", x: bass.AP, out: bass.AP, hw: int):
    nc = tc.nc
    CC = CH // P          # channel chunks (2)
    KT = hw // P          # k tiles (32)
    NPT = hw // PT        # pixel tiles (8)
    PC = PT // P          # p chunks per pixel tile (4)
    # channels + denominator column, padded to an even moving length
    # (fp32r matmul ISA requires even innermost n_step on src and dst)
    CHA = CH + 2

    with ExitStack() as ctx:
        const = ctx.enter_context(tc.tile_pool(name="const", bufs=1))
        sb = ctx.enter_context(tc.tile_pool(name="sb", bufs=1))

        X = sb.tile([P, CC, hw], F32R, tag="X")
        XT = sb.tile([P, KT, CH], F32, tag="XT")
        Khat = sb.tile([P, KT, CHA], F32R, tag="Khat")
        n2 = sb.tile([P, KT], F32, tag="n2")
        recip_n = sb.tile([P, KT], F32, tag="recip_n")

        ident = const.tile([P, P], F32, tag="ident")
        make_identity(nc, ident)
        # ones column of Khat_aug: makes outT[:, 256] the softmax denominator
        nc.vector.memset(Khat[:, :, CH:CHA].bitcast(F32), 1.0)

        # ---- load X: [256, hw] -> [128, cc, hw] ----
        # cc-inner order so early k-tiles (which need both channel halves)
        # arrive first and transposes can start sooner.
        # First slices small so the first transposes start ASAP.
        bounds = [0, hw // 16, hw // 4, hw // 2, 3 * hw // 4, hw]
        for lo, hi in zip(bounds, bounds[1:]):
            for cc in range(CC):
                nc.sync.dma_start(
                    out=X[:, cc, lo:hi],
                    in_=x[cc * P:(cc + 1) * P, lo:hi].bitcast(F32R),
                )

        # ---- setup: transpose X -> XT; n2 = row sumsq; Khat = XT/|XT| ----
        with tc.tile_pool(name="tpsum", bufs=2, space="PSUM") as tpsum, \
             tc.tile_pool(name="tsq", bufs=2) as tsq:
            for kt in range(KT):
                pt_ = tpsum.tile([P, CH], F32, tag="t")
                for cc in range(CC):
                    nc.tensor.transpose(
                        pt_[:, cc * P:(cc + 1) * P],
                        X[:, cc, kt * P:(kt + 1) * P].bitcast(F32),
                        ident,
                    )
                nc.scalar.copy(XT[:, kt, :], pt_)
                sq = tsq.tile([P, CH], F32, tag="sq")
                nc.scalar.activation(
                    sq,
                    XT[:, kt, :],
                    AF.Square,
                    accum_out=n2[:, kt:kt + 1],
                )
            # recip_n = 1/sqrt(n2)
            nc.scalar.sqrt(n2, n2)
            nc.vector.reciprocal(recip_n, n2)
            # Khat = XT * recip_n (per-partition scalar), rounded to f32r
            with nc.allow_low_precision(reason="f32r matmul operand prep"):
                for kt in range(KT):
                    nc.vector.tensor_scalar_mul(
                        out=Khat[:, kt, 0:CH],
                        in0=XT[:, kt, :],
                        scalar1=recip_n[:, kt:kt + 1],
                    )

        # ---- main: per pixel-tile flash attention ----
        with tc.tile_pool(name="ps", bufs=3, space="PSUM") as ps_pool, \
             tc.tile_pool(name="acc", bufs=1, space="PSUM") as acc_pool, \
             tc.tile_pool(name="ework", bufs=6) as e_pool, \
             tc.tile_pool(name="owork", bufs=8) as o_pool, \
             tc.tile_pool(name="zwork", bufs=8) as z_pool:
            for pt in range(NPT):
                outT_ps = [
                    acc_pool.tile([P, CHA], F32, tag=f"acc{pc}",
                                  name=f"outT_ps{pc}")
                    for pc in range(PC)
                ]

                def mm2_group(kc, e_sb):
                    # outT[p, c] += E[k, p-chunk].T @ Khat_aug[k, :]
                    for pc in range(PC):
                        nc.tensor.matmul(
                            outT_ps[pc],
                            lhsT=e_sb[:, pc * P:(pc + 1) * P],
                            rhs=Khat[:, kc, :],
                            start=(kc == 0),
                            stop=(kc == KT - 1),
                        )

                # Software-pipelined: mm2 for k-chunk kc issues after mm1 of
                # kc+1, giving each exp a full k-step of slack on ACT.
                pending = None
                for kc in range(KT):
                    # scores[k, p] = sum_c X[c, k] * X[c, p]
                    s_ps = ps_pool.tile([P, PT], F32, tag="ps")
                    for cc in range(CC):
                        nc.tensor.matmul(
                            s_ps,
                            lhsT=X[:, cc, kc * P:(kc + 1) * P],
                            rhs=X[:, cc, pt * PT:(pt + 1) * PT],
                            start=(cc == 0),
                            stop=(cc == CC - 1),
                        )
                    # E = exp(recip_n[k] * s)
                    e_sb = e_pool.tile([P, PT], F32R, tag="e")
                    nc.scalar.activation(
                        e_sb, s_ps, AF.Exp, scale=recip_n[:, kc:kc + 1],
                    )
                    if pending is not None:
                        mm2_group(*pending)
                    pending = (kc, e_sb)
                mm2_group(*pending)
                # Epilogue per p-chunk: divide by the fused denominator
                # column and DMA out the transposed result.
                for pc in range(PC):
                    rz_sb = z_pool.tile([P, 1], F32, tag="rz", name=f"rz{pc}")
                    nc.vector.reciprocal(rz_sb, outT_ps[pc][:, CH:CH + 1])
                    o_sb = o_pool.tile([P, CH], F32, tag="o", name=f"o{pc}")
                    nc.vector.tensor_scalar_mul(
                        out=o_sb,
                        in0=outT_ps[pc][:, 0:CH],
                        scalar1=rz_sb,
                    )
                    nc.sync.dma_start(
                        out=out[pt * PT + pc * P: pt * PT + (pc + 1) * P, :],
                        in_=o_sb,
                    )


def _legalize_single_wait(nc: bass.Bass) -> None:
    """The walrus build in this container accepts at most ONE sync-wait per
    instruction ("Too many sync wait commands"); Tile emits instructions with
    one wait per outstanding producer. Hoist extra waits onto injected
    same-engine NOPs placed immediately before the instruction — identical
    blocking semantics, one wait each."""
    for fn in nc.m.functions:
        for bb in fn.blocks:
            new = []
            changed = False
            for inst in bb.instructions:
                if (
                    isinstance(inst, mybir.InstISA)
                    and inst.engine == mybir.EngineType.Pool
                ):
                    # Tail-of-kernel semaphore RANGE_CLEAR on GpSimd; this
                    # walrus build rejects its encoding ("ISA wrong length").
                    # Semaphores are re-initialized by the runtime at
                    # execution start, so the in-kernel clear is redundant.
                    # (DVE InstISA ops — e.g. tensor_tensor_reduce — are real
                    # compute and must be kept.)
                    changed = True
                    continue
                si = inst.sync_info
                if si is not None and si.on_wait is not None and len(si.on_wait) > 1:
                    waits = list(si.on_wait)
                    for j, w in enumerate(waits[:-1]):
                        nop = mybir.InstNoOp(
                            name=f"{inst.name}-xw{j}",
                            engine=inst.engine,
                            sync_info=mybir.SyncInfo(on_wait=[w], on_update=[]),
                            bass_nofuse=True,
                        )
                        new.append(nop)
                    si.on_wait = [waits[-1]]
                    changed = True
                new.append(inst)
            if changed:
                bb.instructions = new


def build_nc(hw: int = 4096, legalize: bool = True) -> bass.Bass:
    nc = bass.Bass()
    x = nc.dram_tensor("x", [CH, hw], F32, kind="ExternalInput")
    # out is stored transposed ([hw, ch]); the host un-transposes.
    out = nc.dram_tensor("out", [hw, CH], F32, kind="ExternalOutput")
    with tile.TileContext(nc) as tc:
        _emit(tc, x[:], out[:], hw)
    if legalize:
        _legalize_single_wait(nc)
    return nc


_nc_cache: dict = {}


def kernel(foreground: np.ndarray) -> np.ndarray:
    fg = np.ascontiguousarray(np.asarray(foreground, dtype=np.float32))
    bs, ch, h, w = fg.shape
    assert bs == N_CORES and ch == CH
    hw = h * w
    if hw not in _nc_cache:
        _nc_cache[hw] = build_nc(hw)
    nc = _nc_cache[hw]
    in_maps = [{"x": fg[i].reshape(ch, hw)} for i in range(bs)]
    res = run_bass_kernel_spmd(nc, in_maps, core_ids=list(range(bs)))
    return np.stack(
        [
            np.asarray(res.results[i]["out"]).T.reshape(ch, h, w)
            for i in range(bs)
        ]
    )


# revision 8
# speedup vs baseline: 1.1779x; 1.0432x over previous
"""Trainium2 Bass kernel for nn_ContextualAttention.

Per sample b (one per NeuronCore):
    X   = foreground[b]               # [256, 4096]  (channels x pixels)
    K   = (X + eps).T, L2-normalized rows          # [4096, 256]
    S   = K @ X                        # [4096(k), 4096(p)] scores
    A   = softmax(S, axis=k)
    out = K.T @ A                      # [256, 4096]

Key structure (per core):
  - mm1 runs in fp8 (e4m3) DoubleRow perf mode: the stationary operand is
    KhatT8 = fp8(64 * X * rn) [128c, 2cc, hw] and the moving operand is
    X8 = fp8(X), contracting all 256 channels in ONE instruction at 2 rows
    per cycle.  The row normalization rn_k = 1/|x_k| is folded into the
    stationary operand, so the later exp needs only a CONSTANT 1/64 scale
    (not per-partition), letting one ACT instruction exp a whole group of
    4 score banks (amortizes ACT's ~350ns fixed overhead).
  - The 64x prescale keeps fp8 khat values out of the subnormal range
    (validated offline: rel err 4.5e-4 vs f32 reference).
  - mm2 is swapped: outT[p, c] += E_chunk.T @ Khat_aug with E stationary
    and Khat_aug the moving operand, augmented with ones columns so that
    column 256 of outT is the softmax denominator Z for free (258 moving
    rows, even for the fp32r ISA restriction).
  - Epilogue: rz = 1/outT[:,256], outT[:, :256] * rz per-partition (DVE),
    DMA to DRAM as out^T [hw, 256]; the host un-transposes.

eps=1e-7 is dropped: its effect on the output is O(1e-7) relative, far
below matmul precision.
"""

import numpy as np
from contextlib import ExitStack

import concourse.bass as bass
import concourse.tile as tile
from concourse import mybir
from concourse.bass_utils import run_bass_kernel_spmd
from concourse.masks import make_identity

F32 = mybir.dt.float32
F32R = mybir.dt.float32r
FP8 = mybir.dt.float8e4
AF = mybir.ActivationFunctionType
ALU = mybir.AluOpType
DR = mybir.MatmulPerfMode.DoubleRow

CH = 256     # channels
P = 128      # partitions
PT = 512     # pixel-tile width (matmul moving dim / psum bank)
GRP = 4      # k-chunks per exp group (4 psum banks per ACT instruction)
N_CORES = 8


def _emit(tc: "tile.TileContext", x: bass.AP, out: bass.AP, hw: int):
    nc = tc.nc
    CC = CH // P          # channel chunks (2)
    KT = hw // P          # k tiles (32)
    NPT = hw // PT        # pixel tiles (8)
    PC = PT // P          # p chunks per pixel tile (4)
    NCH = hw // PT        # setup chunks (8)
    # channels + denominator column, padded to an even moving length
    # (fp32r matmul ISA requires even innermost n_step on src and dst)
    CHA = CH + 2

    with ExitStack() as ctx:
        const = ctx.enter_context(tc.tile_pool(name="const", bufs=1))
        sb = ctx.enter_context(tc.tile_pool(name="sb", bufs=1))

        X = sb.tile([P, CC, hw], F32, tag="X")
        X8 = sb.tile([P, CC, hw], FP8, tag="X8")
        KhatT = sb.tile([P, CC, hw], F32R, tag="KhatT")
        KhatT8 = sb.tile([P, CC, hw], FP8, tag="KhatT8")
        Khat = sb.tile([P, KT, CHA], F32R, tag="Khat")
        rn64 = sb.tile([P, hw], F32, tag="rn64")

        ident = const.tile([P, P], F32, tag="ident")
        identr = const.tile([P, P], F32R, tag="identr")
        ones128 = const.tile([P, P], F32R, tag="ones128")
        make_identity(nc, ident)
        with nc.allow_low_precision(reason="f32r matmul operand prep"):
            nc.vector.tensor_copy(identr, ident)
        nc.vector.memset(ones128.bitcast(F32), 1.0)
        # ones columns of Khat_aug -> fused softmax denominator
        nc.vector.memset(Khat[:, :, CH:CHA].bitcast(F32), 1.0)

        # ---- setup, pipelined in pixel chunks of 512 ----
        # rn64[p] = 64/|x_p|; KhatT8 = fp8(X*rn64); X8 = fp8(X)
        with tc.tile_pool(name="n2ps", bufs=2, space="PSUM") as n2ps, \
             tc.tile_pool(name="xsq", bufs=2) as xsq_pool:
            for c8 in range(NCH):
                lo, hi = c8 * PT, (c8 + 1) * PT
                for cc in range(CC):
                    nc.sync.dma_start(
                        out=X[:, cc, lo:hi],
                        in_=x[cc * P:(cc + 1) * P, lo:hi],
                    )
                sq = xsq_pool.tile([P, CC, PT], F32R, tag="sq")
                # n2 replicated on all partitions via all-ones stationary
                n2 = n2ps.tile([P, PT], F32, tag="n2")
                with nc.allow_low_precision(reason="f32r operand prep"):
                    nc.vector.tensor_tensor(
                        out=sq, in0=X[:, :, lo:hi], in1=X[:, :, lo:hi],
                        op=ALU.mult,
                    )
                for cc in range(CC):
                    nc.tensor.matmul(
                        n2, lhsT=ones128, rhs=sq[:, cc, :],
                        start=(cc == 0), stop=(cc == CC - 1),
                    )
                # rn64 = 1/sqrt(n2/4096) = 64/sqrt(n2)
                nc.scalar.activation(
                    rn64[:, lo:hi], n2, AF.Sqrt, scale=1.0 / 4096.0,
                )
                nc.vector.reciprocal(rn64[:, lo:hi], rn64[:, lo:hi])
                with nc.allow_low_precision(reason="fp8/f32r operand prep"):
                    for cc in range(CC):
                        nc.vector.tensor_tensor(
                            out=KhatT[:, cc, lo:hi], in0=X[:, cc, lo:hi],
                            in1=rn64[:, lo:hi], op=ALU.mult,
                        )
                        nc.scalar.copy(X8[:, cc, lo:hi], X[:, cc, lo:hi])
                        nc.vector.tensor_copy(
                            KhatT8[:, cc, lo:hi], KhatT[:, cc, lo:hi],
                        )

        # ---- Khat_aug[k, c] = KhatT.T / 64 via PE transposes ----
        with tc.tile_pool(name="tpsum", bufs=2, space="PSUM") as tpsum:
            for kt in range(KT):
                pt_ = tpsum.tile([P, CH], F32R, tag="t")
                for cc in range(CC):
                    nc.tensor.transpose(
                        pt_[:, cc * P:(cc + 1) * P],
                        KhatT[:, cc, kt * P:(kt + 1) * P],
                        identr,
                    )
                nc.scalar.activation(
                    Khat[:, kt, 0:CH], pt_, AF.Copy, scale=1.0 / 64.0,
                )

        # ---- main: per pixel-tile flash attention ----
        with tc.tile_pool(name="ps", bufs=1, space="PSUM") as ps_pool, \
             tc.tile_pool(name="acc", bufs=1, space="PSUM") as acc_pool, \
             tc.tile_pool(name="ework", bufs=2) as e_pool, \
             tc.tile_pool(name="owork", bufs=8) as o_pool, \
             tc.tile_pool(name="zwork", bufs=8) as z_pool:
            for pt in range(NPT):
                outT_ps = [
                    acc_pool.tile([P, CHA], F32, tag=f"acc{pc}",
                                  name=f"outT_ps{pc}")
                    for pc in range(PC)
                ]

                def mm2_group(g, e4):
                    # outT[p, c] += E[k, p-chunk].T @ Khat_aug[k, :]
                    for j in range(GRP):
                        kc = g * GRP + j
                        for pc in range(PC):
                            nc.tensor.matmul(
                                outT_ps[pc],
                                lhsT=e4[:, j, pc * P:(pc + 1) * P],
                                rhs=Khat[:, kc, :],
                                start=(kc == 0),
                                stop=(kc == KT - 1),
                            )

                pending = None
                for g in range(KT // GRP):
                    # scores[k, p] = khat_k . x_p  (fp8 DoubleRow, x64)
                    s4 = ps_pool.tile([P, GRP, PT], F32, tag="ps")
                    for j in range(GRP):
                        kc = g * GRP + j
                        nc.tensor.matmul(
                            s4[:, j, :],
                            lhsT=KhatT8[:, :, kc * P:(kc + 1) * P],
                            rhs=X8[:, :, pt * PT:(pt + 1) * PT],
                            start=True, stop=True,
                            perf_mode=DR,
                        )
                    # E = exp(s/64) over the whole 4-bank group
                    e4 = e_pool.tile([P, GRP, PT], F32R, tag="e")
                    nc.scalar.activation(e4, s4, AF.Exp, scale=1.0 / 64.0)
                    if pending is not None:
                        mm2_group(*pending)
                    pending = (g, e4)
                mm2_group(*pending)
                # Epilogue per p-chunk: divide by the fused denominator
                # column and DMA out the transposed result.
                for pc in range(PC):
                    rz_sb = z_pool.tile([P, 1], F32, tag="rz", name=f"rz{pc}")
                    nc.vector.reciprocal(rz_sb, outT_ps[pc][:, CH:CH + 1])
                    o_sb = o_pool.tile([P, CH], F32, tag="o", name=f"o{pc}")
                    nc.vector.tensor_scalar_mul(
                        out=o_sb,
                        in0=outT_ps[pc][:, 0:CH],
                        scalar1=rz_sb,
                    )
                    nc.sync.dma_start(
                        out=out[pt * PT + pc * P: pt * PT + (pc + 1) * P, :],
                        in_=o_sb,
                    )


def _legalize_single_wait(nc: bass.Bass) -> None:
    """The walrus build in this container accepts at most ONE sync-wait per
    instruction ("Too many sync wait commands"); Tile emits instructions with
    one wait per outstanding producer. Hoist extra waits onto injected
    same-engine NOPs placed immediately before the instruction — identical
    blocking semantics, one wait each."""
    for fn in nc.m.functions:
        for bb in fn.blocks:
            new = []
            changed = False
            for inst in bb.instructions:
                if (
                    isinstance(inst, mybir.InstISA)
                    and inst.engine == mybir.EngineType.Pool
                ):
                    # Tail-of-kernel semaphore RANGE_CLEAR on GpSimd; this
                    # walrus build rejects its encoding ("ISA wrong length").
                    # Semaphores are re-initialized by the runtime at
                    # execution start, so the in-kernel clear is redundant.
                    changed = True
                    continue
                si = inst.sync_info
                if si is not None and si.on_wait is not None and len(si.on_wait) > 1:
                    waits = list(si.on_wait)
                    for j, w in enumerate(waits[:-1]):
                        nop = mybir.InstNoOp(
                            name=f"{inst.name}-xw{j}",
                            engine=inst.engine,
                            sync_info=mybir.SyncInfo(on_wait=[w], on_update=[]),
                            bass_nofuse=True,
                        )
                        new.append(nop)
                    si.on_wait = [waits[-1]]
                    changed = True
                new.append(inst)
            if changed:
                bb.instructions = new


def build_nc(hw: int = 4096, legalize: bool = True) -> bass.Bass:
    nc = bass.Bass()
    x = nc.dram_tensor("x", [CH, hw], F32, kind="ExternalInput")
    # out is stored transposed ([hw, ch]); the host un-transposes.
    out = nc.dram_tensor("out", [hw, CH], F32, kind="ExternalOutput")
    with tile.TileContext(nc) as tc:
        _emit(tc, x[:], out[:], hw)
    if legalize:
        _legalize_single_wait(nc)
    return nc


_nc_cache: dict = {}


def kernel(foreground: np.ndarray) -> np.ndarray:
    fg = np.ascontiguousarray(np.asarray(foreground, dtype=np.float32))
    bs, ch, h, w = fg.shape
    assert bs == N_CORES and ch == CH
    hw = h * w
    if hw not in _nc_cache:
        _nc_cache[hw] = build_nc(hw)
    nc = _nc_cache[hw]
    in_maps = [{"x": fg[i].reshape(ch, hw)} for i in range(bs)]
    res = run_bass_kernel_spmd(nc, in_maps, core_ids=list(range(bs)))
    return np.stack(
        [
            np.asarray(res.results[i]["out"]).T.reshape(ch, h, w)
            for i in range(bs)
        ]
    )
